# revision 1
# baseline (speedup 1.0000x reference)
"""Event-driven SSM layer (LIF spiking scan) on 8 TRN2 NeuronCores.

Sharding: data-parallel over batch (B=8 -> 1 batch/core). Per-core scan runs
the 32-step LIF recurrence on [S=256] rows in transposed (channel-major)
layout. Adaptive thresholds need a global spike-mean per step -> one fused
AllReduce of a [128,5] f32 count tile per step.

Math notes:
 - anti-spikes ns = (v < thr) are computed instead of spikes; h = 1 - ns is
   folded in via negated A/C weights plus row-sum constants. The row-sum
   constants live in SHIFTED thresholds (thr' = thr - rowsum) and are added
   back in the membrane reset ((v + rowsum) * ns), so PSUM stays pure-matmul.
 - x@D.T, x@B.T run as bf16 hi/lo split matmuls (3 products), A/C as hi/lo
   against the binary anti-spikes (2 products) -> ~1e-4 absolute accuracy.
 - Issue order: xD matmuls are fed 2 steps ahead of the threshold chain so
   the PE has runnable work while each step's AllReduce is in flight.
"""
import numpy as np
import ml_dtypes

B_, T_FULL, S, DM, DS = 8, 32, 256, 512, 64
KC, MC = DM // 128, DM // 128  # 4, 4
N_CORES = 8
ROWS_GLOBAL = float(B_ * S)
DECAY = float(np.float32(np.exp(np.float64(-1.0 / 2.0))))
ADAPT, BASE_THR, TGT = 0.1, 1.0, 0.1

bf16 = ml_dtypes.bfloat16


def _split(a):
    hi = a.astype(bf16)
    lo = (a - hi.astype(np.float32)).astype(bf16)
    return hi, lo


def _build(T):
    from concourse import bacc, mybir, tile

    nc = bacc.Bacc("TRN2", target_bir_lowering=False, debug=False,
                   num_devices=N_CORES)
    f32, bft = mybir.dt.float32, mybir.dt.bfloat16
    ALU = mybir.AluOpType

    def din(name, shape, dt=bft):
        return nc.dram_tensor(name, shape, dt, kind="ExternalInput").ap()

    xhi_d = din("xhi", [T, KC, 128, S])
    xlo_d = din("xlo", [T, KC, 128, S])
    dthi_d = din("dthi", [KC, 128, DM])
    dtlo_d = din("dtlo", [KC, 128, DM])
    bthi_d = din("bthi", [KC, 128, DS])
    btlo_d = din("btlo", [KC, 128, DS])
    nathi_d = din("nathi", [DS, DS])
    natlo_d = din("natlo", [DS, DS])
    ncthi_d = din("ncthi", [DS, DM])
    nctlo_d = din("nctlo", [DS, DM])
    rs_d = din("rs", [128, MC + 1], f32)  # cols 0..3 rowsum(C) chunks, col 4 rowsum(A)
    out_d = nc.dram_tensor("out", [T, MC, 128, S], bft, kind="ExternalOutput").ap()

    CC = MC + 1
    c_upd = -ADAPT / ROWS_GLOBAL
    b_upd = ADAPT * (1.0 - TGT)

    with tile.TileContext(nc) as tc:
        with tc.tile_pool(name="w", bufs=1) as wp, \
             tc.tile_pool(name="st", bufs=1) as stp, \
             tc.tile_pool(name="io", bufs=4) as iop, \
             tc.tile_pool(name="sm", bufs=2) as smp, \
             tc.tile_pool(name="pso", bufs=2, space="PSUM") as pspo, \
             tc.tile_pool(name="psc", bufs=1, space="PSUM") as pspc, \
             tc.tile_pool(name="pss", bufs=2, space="PSUM") as psps, \
             tc.tile_pool(name="dr", bufs=1, space="DRAM") as drp:

            # ---------- persistent weights ----------
            dthi = [wp.tile([128, DM], bft, name=f"dthi{k}") for k in range(KC)]
            dtlo = [wp.tile([128, DM], bft, name=f"dtlo{k}") for k in range(KC)]
            bthi = [wp.tile([128, DS], bft, name=f"bthi{k}") for k in range(KC)]
            btlo = [wp.tile([128, DS], bft, name=f"btlo{k}") for k in range(KC)]
            nathi = wp.tile([DS, DS], bft, name="nathi")
            natlo = wp.tile([DS, DS], bft, name="natlo")
            ncthi = wp.tile([DS, DM], bft, name="ncthi")
            nctlo = wp.tile([DS, DM], bft, name="nctlo")
            rs = wp.tile([128, CC], f32, name="rs")

            for k in range(KC):
                nc.sync.dma_start(out=dthi[k][:, :], in_=dthi_d[k])
                nc.sync.dma_start(out=dtlo[k][:, :], in_=dtlo_d[k])
                nc.sync.dma_start(out=bthi[k][:, :], in_=bthi_d[k])
                nc.sync.dma_start(out=btlo[k][:, :], in_=btlo_d[k])
            nc.sync.dma_start(out=nathi[:, :], in_=nathi_d[:, :])
            nc.sync.dma_start(out=natlo[:, :], in_=natlo_d[:, :])
            nc.sync.dma_start(out=ncthi[:, :], in_=ncthi_d[:, :])
            nc.sync.dma_start(out=nctlo[:, :], in_=nctlo_d[:, :])
            nc.sync.dma_start(out=rs[:, :], in_=rs_d[:, :])

            # ---------- persistent state ----------
            sv = stp.tile([DS, S], f32, name="sv")
            ov = stp.tile([128, MC * S], f32, name="ov")
            thr = stp.tile([128, CC], f32, name="thr")  # shifted: thr - rowsum
            nc.vector.memset(sv[:, :], 0.0)
            nc.vector.memset(ov[:, :], 0.0)
            # thr' = BASE_THR - rs
            nc.vector.tensor_scalar(thr[:, :], rs[:, :], -1.0, BASE_THR,
                                    ALU.mult, ALU.add)

            ari = [drp.tile([128, CC], f32, name=f"ari{t}") for t in range(T)]
            aro = [drp.tile([128, CC], f32, name=f"aro{t}",
                            addr_space="Shared") for t in range(T)]

            xs, pos, psss, nhs, tmps = {}, {}, {}, {}, {}

            def feed_out(t):
                xh = iop.tile([128, KC * S], bft, name=f"xh{t}", tag="xh")
                xl = iop.tile([128, KC * S], bft, name=f"xl{t}", tag="xl")
                for k in range(KC):
                    nc.sync.dma_start(out=xh[:, k * S:(k + 1) * S], in_=xhi_d[t, k])
                    nc.sync.dma_start(out=xl[:, k * S:(k + 1) * S], in_=xlo_d[t, k])
                xs[t] = (xh, xl)
                po = pspo.tile([128, MC * S], f32, name=f"po{t}", tag="po")
                for m in range(MC):
                    pom = po[:, m * S:(m + 1) * S]
                    # start=True zeroes the whole 2KB PSUM bank, so only the
                    # bank-first chunk (m=0 for bank A, m=2 for bank B) may
                    # carry it; odd chunks accumulate onto the cleared bank.
                    first = (m % 2 == 0)
                    n = 0
                    for k in range(KC):
                        xhk, xlk = xh[:, k * S:(k + 1) * S], xl[:, k * S:(k + 1) * S]
                        dh = dthi[k][:, m * 128:(m + 1) * 128]
                        dl = dtlo[k][:, m * 128:(m + 1) * 128]
                        for lhsT, rhs in ((dh, xhk), (dh, xlk), (dl, xhk)):
                            n += 1
                            nc.tensor.matmul(pom, lhsT=lhsT, rhs=rhs, start=first,
                                             stop=(n == 3 * KC),
                                             skip_group_check=True)
                            first = False
                # evict xD to SBUF on ScalarE -> po slot frees without waiting
                # for the threshold chain, so the xD feed runs ahead freely
                xd = smp.tile([128, MC * S], f32, name=f"xd{t}", tag="xd", bufs=3)
                nc.scalar.activation(xd[:, :], po[:, :],
                                     mybir.ActivationFunctionType.Identity)
                pos[t] = xd

            def feed_state(t):
                xh, xl = xs[t]
                pss = psps.tile([DS, S], f32, name=f"pss{t}", tag="pss")
                psss[t] = pss
                prods = []
                for k in range(KC):
                    xhk, xlk = xh[:, k * S:(k + 1) * S], xl[:, k * S:(k + 1) * S]
                    prods += [(bthi[k], xhk), (bthi[k], xlk), (btlo[k], xhk)]
                for i, (lhsT, rhs) in enumerate(prods):
                    nc.tensor.matmul(pss[:, :], lhsT=lhsT[:, :], rhs=rhs,
                                     start=(i == 0),
                                     stop=(t == 0 and i == len(prods) - 1),
                                     skip_group_check=True)

            def chain(t):
                xh, xl = xs.pop(t)
                pss, po = psss.pop(t), pos.pop(t)
                # -- state: finish matmul group --
                if t > 0:
                    nhp = nhs[t - 1]
                    nc.tensor.matmul(pss[:, :], lhsT=nathi[:, :], rhs=nhp[:, :],
                                     start=False, stop=False, skip_group_check=True)
                    nc.tensor.matmul(pss[:, :], lhsT=natlo[:, :], rhs=nhp[:, :],
                                     start=False, stop=True, skip_group_check=True)

                vs = smp.tile([DS, S], f32, name=f"vs{t}", tag="vs")
                nc.vector.scalar_tensor_tensor(
                    out=vs[:, :], in0=sv[:, :], scalar=DECAY, in1=pss[:, :],
                    op0=ALU.mult, op1=ALU.add)

                cnt = smp.tile([128, CC], f32, name=f"cnt{t}", tag="cnt")
                nc.gpsimd.memset(cnt[DS:128, MC:CC], 0.0)
                nh = smp.tile([DS, S], bft, name=f"nh{t}", tag="nh")
                nhs[t] = nh
                s_thr = thr[0:DS, MC:CC] if t > 0 else 1.0
                nc.vector.tensor_scalar(
                    nh[:, :], vs[:, :], s_thr, None, ALU.is_lt, ALU.add,
                    accum_out=cnt[0:DS, MC:CC])
                nc.vector.scalar_tensor_tensor(
                    out=sv[:, :], in0=vs[:, :],
                    scalar=(rs[0:DS, MC:CC] if t > 0 else 0.0), in1=nh[:, :],
                    op0=ALU.add, op1=ALU.mult)

                # -- output stage --
                if t == 0:
                    tmp = smp.tile([128, MC * S], f32, name="tmp0", tag="tmp")
                    nc.vector.scalar_tensor_tensor(
                        out=tmp[:, :], in0=ov[:, :], scalar=DECAY, in1=po[:, :],
                        op0=ALU.mult, op1=ALU.add)
                else:
                    tmp = tmps.pop(t)

                pc = pspc.tile([128, MC * S], f32, name=f"pc{t}", tag="pc")
                for m in range(MC):
                    pcm = pc[:, m * S:(m + 1) * S]
                    nc.tensor.matmul(pcm, lhsT=ncthi[:, m * 128:(m + 1) * 128],
                                     rhs=nh[:, :], start=(m % 2 == 0), stop=False,
                                     skip_group_check=True)
                    nc.tensor.matmul(pcm, lhsT=nctlo[:, m * 128:(m + 1) * 128],
                                     rhs=nh[:, :], start=False, stop=True,
                                     skip_group_check=True)

                vo = smp.tile([128, MC * S], f32, name=f"vo{t}", tag="vo")
                ns = smp.tile([128, MC * S], bft, name=f"ns{t}", tag="ns")
                nc.vector.tensor_tensor(out=vo[:, :], in0=tmp[:, :],
                                        in1=pc[:, :], op=ALU.add)
                for m in range(MC):
                    sl = slice(m * S, (m + 1) * S)
                    nc.vector.tensor_scalar(
                        ns[:, sl], vo[:, sl], thr[:, m:m + 1], None,
                        ALU.is_lt, ALU.add, accum_out=cnt[:, m:m + 1])
                for m in range(MC):
                    sl = slice(m * S, (m + 1) * S)
                    nc.vector.scalar_tensor_tensor(
                        out=ov[:, sl], in0=vo[:, sl], scalar=rs[:, m:m + 1],
                        in1=ns[:, sl], op0=ALU.add, op1=ALU.mult)

                for m in range(MC):
                    nc.scalar.dma_start(out=out_d[t, m], in_=ns[:, m * S:(m + 1) * S])

                # -- fused threshold all-reduce --
                nc.gpsimd.dma_start(out=ari[t][:, :], in_=cnt[:, :])
                nc.gpsimd.collective_compute(
                    "AllReduce", ALU.add,
                    replica_groups=[list(range(N_CORES))],
                    ins=[ari[t][:, :]], outs=[aro[t][:, :]])
                # next step's decay*ov + xd: issued BEFORE the AR-dependent
                # thr ops so the in-order DVE queue does it during the flight
                if t + 1 in pos:
                    tmpn = smp.tile([128, MC * S], f32, name=f"tmp{t+1}", tag="tmp")
                    nc.vector.scalar_tensor_tensor(
                        out=tmpn[:, :], in0=ov[:, :], scalar=DECAY,
                        in1=pos[t + 1][:, :], op0=ALU.mult, op1=ALU.add)
                    tmps[t + 1] = tmpn
                gs = smp.tile([128, CC], f32, name=f"gs{t}", tag="gs")
                dl_t = smp.tile([128, CC], f32, name=f"dl{t}", tag="dl")
                nc.gpsimd.dma_start(out=gs[:, :], in_=aro[t][:, :])
                nc.vector.tensor_scalar(dl_t[:, :], gs[:, :], c_upd, b_upd,
                                        ALU.mult, ALU.add)
                nc.vector.tensor_tensor(out=thr[:, :], in0=thr[:, :],
                                        in1=dl_t[:, :], op=ALU.add)
                nhs.pop(t - 1, None)

            for i in range(T + 2):
                if i >= 2:
                    chain(i - 2)
                if i < T:
                    feed_out(i)
                if 1 <= i <= T:
                    feed_state(i - 1)

    nc.compile()
    return nc


_NC_CACHE = {}


def _np_fallback(x, A, B, C, D):
    """Exact numpy mirror of the reference, incl. the inactive branch.
    Only used if some step has no positive input (never for randn x)."""
    decay = np.float32(np.exp(np.float64(-1.0 / 2.0)))
    Bz = x.shape[0]
    h = np.zeros((Bz, S, DS), np.float32)
    sv = np.zeros_like(h)
    ov = np.zeros((Bz, S, DM), np.float32)
    s_thr = np.full(DS, BASE_THR, np.float32)
    o_thr = np.full(DM, BASE_THR, np.float32)
    outs = []
    for t in range(x.shape[1]):
        xt = x[:, t]
        st = h @ A.T
        if (xt > 0).any():
            vp = sv * decay + st + xt @ B.T
            sp = (vp >= s_thr).astype(np.float32)
            h, sv = sp, vp * (1 - sp)
            s_thr = s_thr + np.float32(ADAPT) * (sp.mean((0, 1)) - np.float32(TGT))
            vo = ov * decay + h @ C.T + xt @ D.T
            so = (vo >= o_thr).astype(np.float32)
            ov = vo * (1 - so)
            o_thr = o_thr + np.float32(ADAPT) * (so.mean((0, 1)) - np.float32(TGT))
            outs.append(so)
        else:
            vp = sv * decay + st
            sp = (vp >= s_thr).astype(np.float32)
            h, sv = sp, vp * (1 - sp)
            s_thr = s_thr + np.float32(ADAPT) * (sp.mean((0, 1)) - np.float32(TGT))
            outs.append(np.zeros_like(ov))
    return np.stack(outs, axis=1)


def kernel(x, A, B, C, D, T=None):
    from concourse.bass_utils import run_bass_kernel_spmd

    x = np.asarray(x, dtype=np.float32)
    A = np.asarray(A, dtype=np.float32)
    B = np.asarray(B, dtype=np.float32)
    C = np.asarray(C, dtype=np.float32)
    D = np.asarray(D, dtype=np.float32)
    T = T or x.shape[1]

    if not (x.reshape(x.shape[0], x.shape[1], -1) > 0).any(axis=(0, 2)).all():
        return _np_fallback(x, A, B, C, D)

    if T not in _NC_CACHE:
        _NC_CACHE[T] = _build(T)
    nc = _NC_CACHE[T]

    dthi, dtlo = _split(D.T.reshape(KC, 128, DM))
    bthi, btlo = _split(B.T.reshape(KC, 128, DS))
    nathi, natlo = _split((-A).T.copy())
    ncthi, nctlo = _split((-C).T.copy())
    rs = np.zeros((128, MC + 1), np.float32)
    rs[:, :MC] = C.sum(axis=1, dtype=np.float32).reshape(MC, 128).T
    rs[:DS, MC] = A.sum(axis=1, dtype=np.float32)

    shared = dict(dthi=dthi, dtlo=dtlo, bthi=bthi, btlo=btlo,
                  nathi=nathi, natlo=natlo, ncthi=ncthi, nctlo=nctlo, rs=rs)

    in_maps = []
    for b in range(N_CORES):
        xt = np.ascontiguousarray(x[b, :T].transpose(0, 2, 1))  # [T, DM, S]
        xhi, xlo = _split(xt.reshape(T, KC, 128, S))
        in_maps.append({"xhi": xhi, "xlo": xlo, **shared})

    res = run_bass_kernel_spmd(nc, in_maps, core_ids=list(range(N_CORES)),
                               trace=bool(__import__("os").environ.get("KTRACE")))
    kernel.last_result = res

    out = np.empty((B_, T, S, DM), dtype=np.float32)
    for b in range(N_CORES):
        ns = res.results[b]["out"].astype(np.float32)  # [T, MC, 128, S]
        out[b] = (1.0 - ns).reshape(T, DM, S).transpose(0, 2, 1)
    return out



# revision 4
# speedup vs baseline: 2.0798x; 2.0798x over previous
"""Event-driven SSM layer (LIF spiking scan) on 8 TRN2 NeuronCores.

Sharding: data-parallel over batch (B=8 -> 1 batch/core). Per-core scan runs
the 32-step LIF recurrence on [S=256] rows in transposed (channel-major)
layout. Adaptive thresholds need a global spike-mean per step -> one fused
AllReduce of a [128,5] f32 count tile per step.

Math notes:
 - anti-spikes ns = (v < thr) are computed instead of spikes; h = 1 - ns is
   folded in via negated A/C weights plus row-sum constants. The row-sum
   constants live in SHIFTED thresholds (thr' = thr - rowsum) and are added
   back in the membrane reset ((v + rowsum) * ns), so PSUM stays pure-matmul.
 - x@D.T, x@B.T run as bf16 hi/lo split matmuls (3 products), A/C as hi/lo
   against the binary anti-spikes (2 products) -> ~1e-4 absolute accuracy.
 - Issue order: xD matmuls are fed 2 steps ahead of the threshold chain so
   the PE has runnable work while each step's AllReduce is in flight.
"""
import numpy as np
import ml_dtypes

B_, T_FULL, S, DM, DS = 8, 32, 256, 512, 64
KC, MC = DM // 128, DM // 128  # 4, 4
N_CORES = 8
ROWS_GLOBAL = float(B_ * S)
DECAY = float(np.float32(np.exp(np.float64(-1.0 / 2.0))))
ADAPT, BASE_THR, TGT = 0.1, 1.0, 0.1

bf16 = ml_dtypes.bfloat16


def _split(a):
    hi = a.astype(bf16)
    lo = (a - hi.astype(np.float32)).astype(bf16)
    return hi, lo


def _build(T):
    from concourse import bacc, mybir, tile

    nc = bacc.Bacc("TRN2", target_bir_lowering=False, debug=False,
                   num_devices=N_CORES)
    f32, bft = mybir.dt.float32, mybir.dt.bfloat16
    ALU = mybir.AluOpType

    def din(name, shape, dt=bft):
        return nc.dram_tensor(name, shape, dt, kind="ExternalInput").ap()

    xhi_d = din("xhi", [T, KC, 128, S])
    xlo_d = din("xlo", [T, KC, 128, S])
    dthi_d = din("dthi", [KC, 128, DM])
    dtlo_d = din("dtlo", [KC, 128, DM])
    bthi_d = din("bthi", [KC, 128, DS])
    btlo_d = din("btlo", [KC, 128, DS])
    nathi_d = din("nathi", [DS, DS])
    natlo_d = din("natlo", [DS, DS])
    ncthi_d = din("ncthi", [DS, DM])
    nctlo_d = din("nctlo", [DS, DM])
    rs_d = din("rs", [128, MC + 1], f32)  # cols 0..3 rowsum(C) chunks, col 4 rowsum(A)
    out_d = nc.dram_tensor("out", [T, MC, 128, S], bft, kind="ExternalOutput").ap()

    CC = MC + 1
    c_upd = -ADAPT / ROWS_GLOBAL
    b_upd = ADAPT * (1.0 - TGT)

    with tile.TileContext(nc) as tc:
        with tc.tile_pool(name="w", bufs=1) as wp, \
             tc.tile_pool(name="st", bufs=1) as stp, \
             tc.tile_pool(name="io", bufs=4) as iop, \
             tc.tile_pool(name="sm", bufs=2) as smp, \
             tc.tile_pool(name="pso", bufs=2, space="PSUM") as pspo, \
             tc.tile_pool(name="psc", bufs=1, space="PSUM") as pspc, \
             tc.tile_pool(name="pss", bufs=2, space="PSUM") as psps, \
             tc.tile_pool(name="dr", bufs=1, space="DRAM") as drp:

            # ---------- persistent weights ----------
            dthi = [wp.tile([128, DM], bft, name=f"dthi{k}") for k in range(KC)]
            dtlo = [wp.tile([128, DM], bft, name=f"dtlo{k}") for k in range(KC)]
            bthi = [wp.tile([128, DS], bft, name=f"bthi{k}") for k in range(KC)]
            btlo = [wp.tile([128, DS], bft, name=f"btlo{k}") for k in range(KC)]
            nathi = wp.tile([DS, DS], bft, name="nathi")
            natlo = wp.tile([DS, DS], bft, name="natlo")
            ncthi = wp.tile([DS, DM], bft, name="ncthi")
            nctlo = wp.tile([DS, DM], bft, name="nctlo")
            rs = wp.tile([128, CC], f32, name="rs")

            for k in range(KC):
                nc.sync.dma_start(out=dthi[k][:, :], in_=dthi_d[k])
                nc.sync.dma_start(out=dtlo[k][:, :], in_=dtlo_d[k])
                nc.sync.dma_start(out=bthi[k][:, :], in_=bthi_d[k])
                nc.sync.dma_start(out=btlo[k][:, :], in_=btlo_d[k])
            nc.sync.dma_start(out=nathi[:, :], in_=nathi_d[:, :])
            nc.sync.dma_start(out=natlo[:, :], in_=natlo_d[:, :])
            nc.sync.dma_start(out=ncthi[:, :], in_=ncthi_d[:, :])
            nc.sync.dma_start(out=nctlo[:, :], in_=nctlo_d[:, :])
            nc.sync.dma_start(out=rs[:, :], in_=rs_d[:, :])

            # ---------- persistent state ----------
            sv = stp.tile([DS, S], f32, name="sv")
            ov = stp.tile([128, MC * S], f32, name="ov")
            thr = stp.tile([128, CC], f32, name="thr")  # shifted: thr - rowsum
            nc.vector.memset(sv[:, :], 0.0)
            nc.vector.memset(ov[:, :], 0.0)
            # thr' = BASE_THR - rs
            nc.vector.tensor_scalar(thr[:, :], rs[:, :], -1.0, BASE_THR,
                                    ALU.mult, ALU.add)

            ari = [drp.tile([128, CC], f32, name=f"ari{t}") for t in range(T)]
            aro = [drp.tile([128, CC], f32, name=f"aro{t}",
                            addr_space="Shared") for t in range(T)]

            xs, pos, psss, nhs, tmps = {}, {}, {}, {}, {}

            def feed_out(t):
                xh = iop.tile([128, KC * S], bft, name=f"xh{t}", tag="xh")
                xl = iop.tile([128, KC * S], bft, name=f"xl{t}", tag="xl")
                for k in range(KC):
                    nc.sync.dma_start(out=xh[:, k * S:(k + 1) * S], in_=xhi_d[t, k])
                    nc.sync.dma_start(out=xl[:, k * S:(k + 1) * S], in_=xlo_d[t, k])
                xs[t] = (xh, xl)
                po = pspo.tile([128, MC * S], f32, name=f"po{t}", tag="po")
                for m in range(MC):
                    pom = po[:, m * S:(m + 1) * S]
                    # start=True zeroes the whole 2KB PSUM bank, so only the
                    # bank-first chunk (m=0 for bank A, m=2 for bank B) may
                    # carry it; odd chunks accumulate onto the cleared bank.
                    first = (m % 2 == 0)
                    n = 0
                    for k in range(KC):
                        xhk, xlk = xh[:, k * S:(k + 1) * S], xl[:, k * S:(k + 1) * S]
                        dh = dthi[k][:, m * 128:(m + 1) * 128]
                        dl = dtlo[k][:, m * 128:(m + 1) * 128]
                        for lhsT, rhs in ((dh, xhk), (dh, xlk), (dl, xhk)):
                            n += 1
                            nc.tensor.matmul(pom, lhsT=lhsT, rhs=rhs, start=first,
                                             stop=(n == 3 * KC),
                                             skip_group_check=True)
                            first = False
                # evict xD to SBUF on ScalarE -> po slot frees without waiting
                # for the threshold chain, so the xD feed runs ahead freely
                xd = smp.tile([128, MC * S], f32, name=f"xd{t}", tag="xd", bufs=3)
                nc.scalar.activation(xd[:, :], po[:, :],
                                     mybir.ActivationFunctionType.Identity)
                pos[t] = xd

            def feed_state(t):
                xh, xl = xs[t]
                pss = psps.tile([DS, S], f32, name=f"pss{t}", tag="pss")
                psss[t] = pss
                prods = []
                for k in range(KC):
                    xhk, xlk = xh[:, k * S:(k + 1) * S], xl[:, k * S:(k + 1) * S]
                    prods += [(bthi[k], xhk), (bthi[k], xlk), (btlo[k], xhk)]
                for i, (lhsT, rhs) in enumerate(prods):
                    nc.tensor.matmul(pss[:, :], lhsT=lhsT[:, :], rhs=rhs,
                                     start=(i == 0),
                                     stop=(t == 0 and i == len(prods) - 1),
                                     skip_group_check=True)

            def chain(t):
                xh, xl = xs.pop(t)
                pss, po = psss.pop(t), pos.pop(t)
                # -- state: finish matmul group --
                if t > 0:
                    nhp = nhs[t - 1]
                    nc.tensor.matmul(pss[:, :], lhsT=nathi[:, :], rhs=nhp[:, :],
                                     start=False, stop=False, skip_group_check=True)
                    nc.tensor.matmul(pss[:, :], lhsT=natlo[:, :], rhs=nhp[:, :],
                                     start=False, stop=True, skip_group_check=True)

                vs = smp.tile([DS, S], f32, name=f"vs{t}", tag="vs")
                nc.vector.scalar_tensor_tensor(
                    out=vs[:, :], in0=sv[:, :], scalar=DECAY, in1=pss[:, :],
                    op0=ALU.mult, op1=ALU.add)

                cnt = smp.tile([128, CC], f32, name=f"cnt{t}", tag="cnt")
                nc.gpsimd.memset(cnt[DS:128, MC:CC], 0.0)
                nh = smp.tile([DS, S], bft, name=f"nh{t}", tag="nh")
                nhs[t] = nh
                s_thr = thr[0:DS, MC:CC] if t > 0 else 1.0
                nc.vector.tensor_scalar(
                    nh[:, :], vs[:, :], s_thr, None, ALU.is_lt, ALU.add,
                    accum_out=cnt[0:DS, MC:CC])
                nc.vector.scalar_tensor_tensor(
                    out=sv[:, :], in0=vs[:, :],
                    scalar=(rs[0:DS, MC:CC] if t > 0 else 0.0), in1=nh[:, :],
                    op0=ALU.add, op1=ALU.mult)

                # -- output stage --
                if t == 0:
                    tmp = smp.tile([128, MC * S], f32, name="tmp0", tag="tmp")
                    nc.vector.scalar_tensor_tensor(
                        out=tmp[:, :], in0=ov[:, :], scalar=DECAY, in1=po[:, :],
                        op0=ALU.mult, op1=ALU.add)
                else:
                    tmp = tmps.pop(t)

                pc = pspc.tile([128, MC * S], f32, name=f"pc{t}", tag="pc")
                for m in range(MC):
                    pcm = pc[:, m * S:(m + 1) * S]
                    nc.tensor.matmul(pcm, lhsT=ncthi[:, m * 128:(m + 1) * 128],
                                     rhs=nh[:, :], start=(m % 2 == 0), stop=False,
                                     skip_group_check=True)
                    nc.tensor.matmul(pcm, lhsT=nctlo[:, m * 128:(m + 1) * 128],
                                     rhs=nh[:, :], start=False, stop=True,
                                     skip_group_check=True)

                vo = smp.tile([128, MC * S], f32, name=f"vo{t}", tag="vo")
                ns = smp.tile([128, MC * S], bft, name=f"ns{t}", tag="ns")
                nc.vector.tensor_tensor(out=vo[:, :], in0=tmp[:, :],
                                        in1=pc[:, :], op=ALU.add)
                for m in range(MC):
                    sl = slice(m * S, (m + 1) * S)
                    nc.vector.tensor_scalar(
                        ns[:, sl], vo[:, sl], thr[:, m:m + 1], None,
                        ALU.is_lt, ALU.add, accum_out=cnt[:, m:m + 1])
                for m in range(MC):
                    sl = slice(m * S, (m + 1) * S)
                    nc.vector.scalar_tensor_tensor(
                        out=ov[:, sl], in0=vo[:, sl], scalar=rs[:, m:m + 1],
                        in1=ns[:, sl], op0=ALU.add, op1=ALU.mult)

                for m in range(MC):
                    nc.scalar.dma_start(out=out_d[t, m], in_=ns[:, m * S:(m + 1) * S])

                # -- fused threshold all-reduce --
                nc.gpsimd.dma_start(out=ari[t][:, :], in_=cnt[:, :])
                nc.gpsimd.collective_compute(
                    "AllReduce", ALU.add,
                    replica_groups=[list(range(N_CORES))],
                    ins=[ari[t][:, :]], outs=[aro[t][:, :]])
                # next step's decay*ov + xd: issued BEFORE the AR-dependent
                # thr ops so the in-order DVE queue does it during the flight
                if t + 1 in pos:
                    tmpn = smp.tile([128, MC * S], f32, name=f"tmp{t+1}", tag="tmp")
                    nc.vector.scalar_tensor_tensor(
                        out=tmpn[:, :], in0=ov[:, :], scalar=DECAY,
                        in1=pos[t + 1][:, :], op0=ALU.mult, op1=ALU.add)
                    tmps[t + 1] = tmpn
                gs = smp.tile([128, CC], f32, name=f"gs{t}", tag="gs")
                dl_t = smp.tile([128, CC], f32, name=f"dl{t}", tag="dl")
                nc.gpsimd.dma_start(out=gs[:, :], in_=aro[t][:, :])
                nc.vector.tensor_scalar(dl_t[:, :], gs[:, :], c_upd, b_upd,
                                        ALU.mult, ALU.add)
                nc.vector.tensor_tensor(out=thr[:, :], in0=thr[:, :],
                                        in1=dl_t[:, :], op=ALU.add)
                nhs.pop(t - 1, None)

            for i in range(T + 2):
                if i >= 2:
                    chain(i - 2)
                if i < T:
                    feed_out(i)
                if 1 <= i <= T:
                    feed_state(i - 1)

    nc.compile()
    return nc


_NC_CACHE = {}


def _np_fallback(x, A, B, C, D):
    """Exact numpy mirror of the reference, incl. the inactive branch.
    Only used if some step has no positive input (never for randn x)."""
    decay = np.float32(np.exp(np.float64(-1.0 / 2.0)))
    Bz = x.shape[0]
    h = np.zeros((Bz, S, DS), np.float32)
    sv = np.zeros_like(h)
    ov = np.zeros((Bz, S, DM), np.float32)
    s_thr = np.full(DS, BASE_THR, np.float32)
    o_thr = np.full(DM, BASE_THR, np.float32)
    outs = []
    for t in range(x.shape[1]):
        xt = x[:, t]
        st = h @ A.T
        if (xt > 0).any():
            vp = sv * decay + st + xt @ B.T
            sp = (vp >= s_thr).astype(np.float32)
            h, sv = sp, vp * (1 - sp)
            s_thr = s_thr + np.float32(ADAPT) * (sp.mean((0, 1)) - np.float32(TGT))
            vo = ov * decay + h @ C.T + xt @ D.T
            so = (vo >= o_thr).astype(np.float32)
            ov = vo * (1 - so)
            o_thr = o_thr + np.float32(ADAPT) * (so.mean((0, 1)) - np.float32(TGT))
            outs.append(so)
        else:
            vp = sv * decay + st
            sp = (vp >= s_thr).astype(np.float32)
            h, sv = sp, vp * (1 - sp)
            s_thr = s_thr + np.float32(ADAPT) * (sp.mean((0, 1)) - np.float32(TGT))
            outs.append(np.zeros_like(ov))
    return np.stack(outs, axis=1)


def kernel(x, A, B, C, D, T=None):
    from concourse.bass_utils import run_bass_kernel_spmd

    x = np.asarray(x, dtype=np.float32)
    A = np.asarray(A, dtype=np.float32)
    B = np.asarray(B, dtype=np.float32)
    C = np.asarray(C, dtype=np.float32)
    D = np.asarray(D, dtype=np.float32)
    T = T or x.shape[1]

    if not (x.reshape(x.shape[0], x.shape[1], -1) > 0).any(axis=(0, 2)).all():
        return _np_fallback(x, A, B, C, D)

    if T not in _NC_CACHE:
        _NC_CACHE[T] = _build(T)
    nc = _NC_CACHE[T]

    dthi, dtlo = _split(D.T.reshape(KC, 128, DM))
    bthi, btlo = _split(B.T.reshape(KC, 128, DS))
    nathi, natlo = _split((-A).T.copy())
    ncthi, nctlo = _split((-C).T.copy())
    rs = np.zeros((128, MC + 1), np.float32)
    rs[:, :MC] = C.sum(axis=1, dtype=np.float32).reshape(MC, 128).T
    rs[:DS, MC] = A.sum(axis=1, dtype=np.float32)

    shared = dict(dthi=dthi, dtlo=dtlo, bthi=bthi, btlo=btlo,
                  nathi=nathi, natlo=natlo, ncthi=ncthi, nctlo=nctlo, rs=rs)

    in_maps = []
    for b in range(N_CORES):
        xt = np.ascontiguousarray(x[b, :T].transpose(0, 2, 1))  # [T, DM, S]
        xhi, xlo = _split(xt.reshape(T, KC, 128, S))
        in_maps.append({"xhi": xhi, "xlo": xlo, **shared})

    res = run_bass_kernel_spmd(nc, in_maps, core_ids=list(range(N_CORES)),
                               trace=bool(__import__("os").environ.get("KTRACE")))
    kernel.last_result = res

    out = np.empty((B_, T, S, DM), dtype=np.float32)
    for b in range(N_CORES):
        ns = res.results[b]["out"].astype(np.float32)  # [T, MC, 128, S]
        out[b] = (1.0 - ns).reshape(T, DM, S).transpose(0, 2, 1)
    return out

